# revision 25
# baseline (speedup 1.0000x reference)
"""Decode-step multi-head attention with KV cache (DeepSpeed-inference style).

Full shapes (hardcoded per problem spec):
  query/key/value: [16, 1, 2048] f32
  key_cache/value_cache: [16, 16, 4096, 128] f32
  cache_len: scalar int (2048)
Output: [16, 1, 2048] f32

Strategy: data-parallel over batch across 8 NeuronCores (2 batches/core =
32 (batch, head) pairs per core). Per pair, the core streams the K and V
cache slices ([cache_len, 128] each) from HBM, computes scores with fused
multiply+reduce on VectorE (K stays in its natural [k, d] layout), exp via
ScalarE (with fused row-sum for the softmax denominator), and aggregates
V with TensorE matmuls (contraction over the k partition axis). The new
token's score/value is folded in as an extra column / extra matmul. The
softmax denominator is reduced across partitions with a ones-vector
matmul; division happens once at the end in pair-major layout.
"""

import functools
from contextlib import ExitStack

import numpy as np

import concourse.bacc as bacc
import concourse.bass as bass
import concourse.mybir as mybir
import concourse.tile as tile
from concourse import bass_utils

N_CORES = 8
P = 128  # partitions
NEG_BIG = -1e30

# test.py hooks: set TRACE=True before calling kernel() to collect a profile.
TRACE = False
TRACE_KWARGS = {}
LAST_RESULTS = None


def _build_program(bl: int, n_heads: int, max_seq: int, hd: int, cache_len: int):
    """Build + compile the per-core program. bl = local batch count."""
    npairs = bl * n_heads
    assert npairs <= P
    assert hd == P
    nch = cache_len // P          # full 128-row chunks of the cache
    rem = cache_len - nch * P     # remainder rows
    ncht = nch + (1 if rem else 0)
    sm_scale = 1.0 / float(np.sqrt(hd))

    nc = bacc.Bacc("TRN2", target_bir_lowering=False, debug=False)
    f32 = mybir.dt.float32
    f16 = mybir.dt.float16

    kc = nc.dram_tensor("kc", [bl, n_heads, max_seq, hd], f32, kind="ExternalInput").ap()
    vc = nc.dram_tensor("vc", [bl, n_heads, max_seq, hd], f32, kind="ExternalInput").ap()
    q = nc.dram_tensor("q", [npairs, hd], f32, kind="ExternalInput").ap()
    kn = nc.dram_tensor("kn", [npairs, hd], f32, kind="ExternalInput").ap()
    vn = nc.dram_tensor("vn", [npairs, hd], f32, kind="ExternalInput").ap()
    ident = nc.dram_tensor("ident", [P, P], f32, kind="ExternalInput").ap()
    out = nc.dram_tensor("out", [npairs, hd], f32, kind="ExternalOutput").ap()

    with tile.TileContext(nc) as tc, ExitStack() as ctx:
        singles = ctx.enter_context(tc.tile_pool(name="singles", bufs=1))
        kpool = ctx.enter_context(tc.tile_pool(name="kpool", bufs=6))
        vpool = ctx.enter_context(tc.tile_pool(name="vpool", bufs=6))
        ppool = ctx.enter_context(tc.tile_pool(name="ppool", bufs=3))
        stats = ctx.enter_context(tc.tile_pool(name="stats", bufs=6))
        psum_o = ctx.enter_context(tc.tile_pool(name="psum_o", bufs=5, space="PSUM"))
        psum_1 = ctx.enter_context(tc.tile_pool(name="psum_1", bufs=1, space="PSUM"))

        def emit_loads(b, h):
            kt = kpool.tile([P, ncht, hd], f32, tag="kt")
            # V is cast to fp16 during the DMA (SWDGE): halves PE matmul
            # passes; psum accumulation stays fp32.
            vt = vpool.tile([P, ncht, hd], f16, tag="vt")
            if nch:
                kslc = kc[b, h, 0 : nch * P, :].rearrange("(p c) d -> p c d", c=nch)
                vslc = vc[b, h, 0 : nch * P, :].rearrange("(p c) d -> p c d", c=nch)
                nc.sync.dma_start(out=kt[:, :nch, :], in_=kslc)
                nc.gpsimd.dma_start(out=vt[:, :nch, :], in_=vslc)
            if rem:
                # zero V pad first so (p == 0) x garbage cannot produce NaN
                nc.gpsimd.memset(vt[:, nch, :], 0.0)
                nc.sync.dma_start(out=kt[:rem, nch, :], in_=kc[b, h, nch * P : cache_len, :])
                nc.gpsimd.dma_start(out=vt[:rem, nch, :], in_=vc[b, h, nch * P : cache_len, :])
            return kt, vt

        # issue the first pairs' streaming loads before any setup traffic
        PRELOAD = 2
        preloaded = [emit_loads(*divmod(p, n_heads)) for p in range(min(PRELOAD, npairs))]

        ones_col = singles.tile([P, 1], f32)
        nc.vector.memset(ones_col, 1.0)

        ident_sb = singles.tile([P, P], f32)
        nc.sync.dma_start(out=ident_sb, in_=ident)

        kn_all = singles.tile([npairs, hd], f32)
        nc.sync.dma_start(out=kn_all, in_=kn)
        vn_all = singles.tile([npairs, hd], f32)
        nc.sync.dma_start(out=vn_all, in_=vn)
        q_all = singles.tile([npairs, hd], f32)
        nc.sync.dma_start(out=q_all, in_=q)

        # all queries broadcast to every partition, once:
        # q_all_b[j, p, d] = q[p, d]
        q_all_b = singles.tile([P, npairs, hd], f32)
        q_bsrc = bass.AP(tensor=q.tensor, offset=q.offset, ap=[[0, P]] + q.ap)
        nc.gpsimd.dma_start(out=q_all_b, in_=q_bsrc)

        # Softmax denominators, one column per pair (partition 0).
        lrow = psum_1.tile([1, npairs], f32)
        # Unnormalized cache-part outputs, head-dim on partitions, one
        # column per pair.
        out_sb = singles.tile([P, npairs], f32)

        # ---- new-token contribution, batched over all pairs ----
        prod_new = singles.tile([npairs, hd], f32)
        nc.vector.tensor_mul(prod_new, kn_all, q_all)
        s_new = singles.tile([npairs, 1], f32)
        nc.vector.reduce_sum(s_new, prod_new, axis=mybir.AxisListType.X)
        p_new = singles.tile([npairs, 1], f32)
        nc.scalar.activation(
            out=p_new, in_=s_new, func=mybir.ActivationFunctionType.Exp, scale=sm_scale
        )
        # rows 0..npairs-1: p_new[p] * v_new[p]; rest zero
        vns = singles.tile([P, hd], f32)
        nc.vector.memset(vns, 0.0)
        nc.vector.tensor_scalar_mul(vns[:npairs, :], vn_all, p_new)
        vnsT_ps = psum_1.tile([P, P], f32)
        nc.tensor.transpose(vnsT_ps, vns, ident_sb)
        vnsT = singles.tile([P, npairs], f32)
        nc.scalar.copy(vnsT, vnsT_ps[:, :npairs])

        for p in range(npairs):
            b, h = divmod(p, n_heads)

            kt, vt = preloaded[p] if p < len(preloaded) else emit_loads(b, h)

            q_b = q_all_b[:, p, :]

            # scores: s[kpart, c] = sum_d K[k, d] * q[d]
            s_tile = stats.tile([P, ncht], f32, tag="s")
            prod = ppool.tile([P, ncht, hd], f32, tag="prod")
            if rem:
                nc.gpsimd.memset(s_tile[:, nch : nch + 1], NEG_BIG)
            if nch:
                q_bb = bass.AP(
                    tensor=q_b.tensor,
                    offset=q_b.offset,
                    ap=[q_b.ap[0], [0, nch], q_b.ap[1]],
                )
                nc.vector.tensor_mul(prod[:, :nch, :], kt[:, :nch, :], q_bb)
                nc.vector.reduce_sum(
                    s_tile[:, :nch], prod[:, :nch, :], axis=mybir.AxisListType.X
                )
            if rem:
                nc.vector.tensor_mul(prod[:rem, nch, :], kt[:rem, nch, :], q_b[:rem, :])
                nc.vector.reduce_sum(
                    s_tile[:rem, nch : nch + 1],
                    prod[:rem, nch, :],
                    axis=mybir.AxisListType.X,
                )

            # softmax numerator (scores scaled here) + fused per-partition sums
            p_tile = stats.tile([P, ncht], f16, tag="p")
            l_part = stats.tile([P, 1], f32, tag="l")
            nc.scalar.activation(
                out=p_tile,
                in_=s_tile,
                func=mybir.ActivationFunctionType.Exp,
                scale=sm_scale,
                accum_out=l_part,
            )

            # V aggregation: out[d] = sum_k p[k] V[k, d]
            acc = psum_o.tile([P, 1], f32, tag="acc")
            for c in range(ncht):
                nc.tensor.matmul(
                    acc,
                    lhsT=vt[:, c, :],
                    rhs=p_tile[:, c : c + 1],
                    start=(c == 0),
                    stop=(c == ncht - 1),
                )

            # softmax denominator (cache part): sum l_part over partitions
            nc.tensor.matmul(
                lrow[0:1, p : p + 1], lhsT=ones_col, rhs=l_part, start=True, stop=True
            )

            nc.scalar.copy(out_sb[:, p : p + 1], acc)

        # ---- epilogue: add new-token contribution, normalize, emit ----
        out_full = singles.tile([P, npairs], f32)
        nc.vector.tensor_add(out_full, out_sb, vnsT)

        l32 = singles.tile([32, 32], f32)
        nc.vector.memset(l32, 0.0)
        nc.scalar.copy(l32[0:1, :npairs], lrow)
        l32t = singles.tile([32, 32], f32)
        nc.vector.transpose(l32t, l32)
        l_tot = singles.tile([npairs, 1], f32)
        nc.vector.tensor_add(l_tot, l32t[:npairs, 0:1], p_new)
        recip_l = singles.tile([npairs, 1], f32)
        nc.vector.reciprocal(recip_l, l_tot)

        oT = psum_1.tile([npairs, hd], f32)
        nc.tensor.transpose(oT, out_full, ident_sb)

        final_sb = singles.tile([npairs, hd], f32)
        nc.scalar.mul(final_sb, oT, mul=recip_l)
        nc.sync.dma_start(out=out, in_=final_sb)

    nc.compile()
    return nc


@functools.lru_cache(maxsize=4)
def _program(bl, n_heads, max_seq, hd, cache_len):
    return _build_program(bl, n_heads, max_seq, hd, cache_len)


def kernel(query, key, value, key_cache, value_cache, cache_len):
    global LAST_RESULTS
    query = np.asarray(query, dtype=np.float32)
    key = np.asarray(key, dtype=np.float32)
    value = np.asarray(value, dtype=np.float32)
    key_cache = np.asarray(key_cache, dtype=np.float32)
    value_cache = np.asarray(value_cache, dtype=np.float32)
    cache_len = int(cache_len)

    b_sz, q_len, d_model = query.shape
    _, n_heads, max_seq, hd = key_cache.shape
    assert q_len == 1 and d_model == n_heads * hd
    assert b_sz % N_CORES == 0
    bl = b_sz // N_CORES

    prog = _program(bl, n_heads, max_seq, hd, cache_len)

    ident = np.eye(P, dtype=np.float32)
    in_maps = []
    for i in range(N_CORES):
        sl = slice(i * bl, (i + 1) * bl)
        in_maps.append(
            {
                "kc": np.ascontiguousarray(key_cache[sl]),
                "vc": np.ascontiguousarray(value_cache[sl]),
                "q": np.ascontiguousarray(query[sl]).reshape(bl * n_heads, hd),
                "kn": np.ascontiguousarray(key[sl]).reshape(bl * n_heads, hd),
                "vn": np.ascontiguousarray(value[sl]).reshape(bl * n_heads, hd),
                "ident": ident,
            }
        )

    try:
        res = bass_utils.run_bass_kernel_spmd(
            prog, in_maps, core_ids=list(range(N_CORES)), trace=TRACE, **TRACE_KWARGS
        )
    except Exception:
        # A previously crashed NeuronCore can leave the first execution
        # attempt failing with a transient runtime error; retry once.
        res = bass_utils.run_bass_kernel_spmd(
            prog, in_maps, core_ids=list(range(N_CORES)), trace=TRACE, **TRACE_KWARGS
        )
    LAST_RESULTS = res
    outs = [res.results[i]["out"].reshape(bl, q_len, d_model) for i in range(N_CORES)]
    return np.concatenate(outs, axis=0)
